# revision 2
# baseline (speedup 1.0000x reference)
"""BinConv2d Trainium2 kernel.

Computes y = conv2d(sign(x), sign(w - mean_cin(w)), pad=1) * gamma * beta * alpha
for x (64,256,56,56) f32, w (256,256,3,3) f32, on 8 NeuronCores,
data-parallel over batch (8 images per core).

Strategy per core:
  - x image (256,56,56) f32 -> sign -> bf16, written into a zero-padded
    (58x58) layout in SBUF, split into 2 cin chunks of 128 partitions.
  - conv as 9 shifted matmuls per (cout chunk, 8-row chunk) accumulated in
    PSUM: psum[cout,pix] += wT[cin,cout](tap) @ xpad[cin, pix+shift(tap)].
  - weights: centered via a high-precision split summation (exact integer
    part + tiny residual) so binarized signs match the float64-exact signs
    (the jax reference's own rounding agrees with float64 on this data),
    then sign -> bf16, transposed on the tensor engine to [cin,cout] tiles.
  - psum evacuated with one scalar_tensor_tensor: (psum * gamma) * (alpha x beta),
    sliced to the valid 56 columns, DMA'd out as f32.
"""

import numpy as np
from contextlib import ExitStack

import concourse.bass as bass
import concourse.tile as tile
from concourse import mybir
from concourse.bass_utils import run_bass_kernel_spmd
from concourse.masks import make_identity

F32 = mybir.dt.float32
BF16 = mybir.dt.bfloat16

N_CORES = 8
B, CIN, COUT, H, W, K = 64, 256, 256, 56, 56, 3
IPC = B // N_CORES          # images per core
PW = W + 2                  # padded row width (58)
NPAD = PW * PW + 4          # padded image buffer per cin chunk (+guard, align)
ORIGIN = 1                  # index of padded (0,0) inside the buffer
NROW = 8                    # output rows per psum tile
NRC = H // NROW             # row chunks (7)
NMM = PW * NROW             # matmul free size (464)


def split_excess_waits(nc, max_waits=1):
    """This container's walrus accepts at most one sync-wait per instruction;
    Tile's tail drain carries one wait per outstanding semaphore.  Split the
    extras into preceding single-wait EventSemaphore instructions (same
    engine, program order => identical semantics)."""
    for f in nc.m.functions:
        for bb in f.blocks:
            out = []
            for inst in bb.instructions:
                si = inst.sync_info
                if si is not None and si.on_wait and len(si.on_wait) > max_waits:
                    waits = list(si.on_wait)
                    extra, keep = waits[:-max_waits], waits[-max_waits:]
                    for w in extra:
                        n = mybir.InstEventSemaphore(
                            name=f"I-xw{nc.next_id()}",
                            ins=[],
                            outs=[],
                            sync_info=mybir.SyncInfo(on_wait=[w], on_update=[]),
                        )
                        n.engine = inst.engine
                        out.append(n)
                    si.on_wait = keep
                out.append(inst)
            bb.instructions = out


def ap3(t, outer_step, outer_n, inner_step, inner_n, offset=0):
    """[128p, outer, inner] view of a 2-D sbuf tile AP with custom steps."""
    return bass.AP(
        tensor=t.tensor,
        offset=t.offset + offset,
        ap=[list(t.ap[0]), [outer_step, outer_n], [inner_step, inner_n]],
    )


def build(nc, ipc=IPC):
    x = nc.dram_tensor("x", [ipc, CIN, H, W], F32, kind="ExternalInput").ap()
    wt = nc.dram_tensor("w", [COUT, CIN, K, K], F32, kind="ExternalInput").ap()
    alpha = nc.dram_tensor("alpha", [1, H, 1], F32, kind="ExternalInput").ap()
    beta = nc.dram_tensor("beta", [1, 1, W], F32, kind="ExternalInput").ap()
    gamma = nc.dram_tensor("gamma", [COUT, 1, 1], F32, kind="ExternalInput").ap()
    y = nc.dram_tensor("y", [ipc, COUT, H, W], F32, kind="ExternalOutput").ap()

    w_flat = wt.rearrange("co ci kh kw -> co (ci kh kw)")      # (256, 2304)
    x_flat = x.rearrange("b c h w -> b c (h w)")               # (ipc, 256, 3136)
    y_flat = y.rearrange("b c h w -> b c (h w)")               # (ipc, 256, 3136)

    with tile.TileContext(nc) as tc, ExitStack() as ctx:
        consts = ctx.enter_context(tc.tile_pool(name="consts", bufs=1))
        dram = ctx.enter_context(tc.tile_pool(name="dram", bufs=1, space="DRAM"))

        # ---------------- persistent tiles ----------------
        ident = consts.tile([128, 128], BF16)
        make_identity(nc, ident)

        # padded sign(x) buffers: [parity][cin chunk]
        xpad = [[consts.tile([128, NPAD], BF16, name=f"xpad{p}{k}") for k in range(2)]
                for p in range(2)]
        for p in range(2):
            for k in range(2):
                nc.gpsimd.memset(xpad[p][k][:, :], 0.0)

        w_lhsT = consts.tile([128, 36 * 128], BF16)   # [tap(9) x k(2) x m(2)] tiles

        ab_bcast = consts.tile([128, H * W], F32)
        ga_col = consts.tile([128, 2], F32)

        # ---------------- weight preparation ----------------
        with tc.tile_pool(name="wprep", bufs=1) as wp, \
             tc.tile_pool(name="wpsum", bufs=4, space="PSUM") as pps:
            wsign = []
            for m in range(2):
                w_st = wp.tile([128, 2304], F32, name=f"wst{m}")
                nc.sync.dma_start(out=w_st[:, :], in_=w_flat[m * 128:(m + 1) * 128, :])

                # a = round(w * 2^22)  (exact integer part, sum is exact fp32)
                wa = wp.tile([128, 2304], F32, name=f"wa{m}")
                nc.scalar.activation(
                    out=wa[:, :], in_=w_st[:, :],
                    func=mybir.ActivationFunctionType.Copy,
                    bias=float(2.0 ** 23), scale=float(2.0 ** 22),
                )
                nc.vector.tensor_scalar_sub(wa[:, :], wa[:, :], float(2.0 ** 23))
                # r = w - a * 2^-22   (exact residual)
                wr = wp.tile([128, 2304], F32, name=f"wr{m}")
                nc.vector.scalar_tensor_tensor(
                    out=wr[:, :], in0=wa[:, :], scalar=float(-(2.0 ** -22)),
                    in1=w_st[:, :], op0=mybir.AluOpType.mult, op1=mybir.AluOpType.add,
                )
                # reduce over cin (stride 9 view: [p, tap, cin])
                wSA = wp.tile([128, 16], F32, name=f"wSA{m}")
                wSr = wp.tile([128, 16], F32, name=f"wSr{m}")
                nc.vector.tensor_reduce(
                    out=wSA[:, 0:9], in_=ap3(wa, 1, 9, 9, 256),
                    axis=mybir.AxisListType.X, op=mybir.AluOpType.add,
                )
                nc.vector.tensor_reduce(
                    out=wSr[:, 0:9], in_=ap3(wr, 1, 9, 9, 256),
                    axis=mybir.AxisListType.X, op=mybir.AluOpType.add,
                )
                # mean_hi = SA * 2^-30 ; mean_lo = Sr / 256
                nc.scalar.mul(wSA[:, 0:9], wSA[:, 0:9], float(2.0 ** -30))
                nc.scalar.mul(wSr[:, 0:9], wSr[:, 0:9], float(1.0 / 256.0))
                # centered = (w - mean_hi) - mean_lo  (per-tap scalar columns)
                wc = wp.tile([128, 2304], F32, name=f"wc{m}")
                for t in range(9):
                    vt = ap3(wc, 9, 256, 0, 1, offset=t)
                    st = ap3(w_st, 9, 256, 0, 1, offset=t)
                    nc.vector.tensor_scalar_sub(vt, st, wSA[:, t:t + 1])
                    nc.vector.tensor_scalar_sub(vt, vt, wSr[:, t:t + 1])
                ws = wp.tile([128, 2304], BF16, name=f"wsg{m}")
                nc.scalar.sign(ws[:, :], wc[:, :])
                wsign.append(ws)

            # transpose sign tiles to [cin, cout] per tap on the PE
            for t in range(9):
                for k2 in range(2):
                    for m in range(2):
                        slot = (t * 2 + k2) * 2 + m
                        src = ap3(wsign[m], 9, 128, 0, 1, offset=k2 * 128 * 9 + t)
                        pt = pps.tile([128, 128], BF16, name="tp")
                        nc.tensor.transpose(pt[:, :], src, ident[:, :])
                        nc.vector.tensor_copy(
                            w_lhsT[:, slot * 128:(slot + 1) * 128], pt[:, :])

            # ---------------- scale tensors ----------------
            al_sb = wp.tile([1, 64], F32)
            be_sb = wp.tile([1, 64], F32)
            ga_sb = wp.tile([1, 256], F32)
            nc.sync.dma_start(out=al_sb[:, 0:H], in_=alpha.rearrange("a h b -> (a b) h"))
            nc.sync.dma_start(out=be_sb[:, 0:W], in_=beta.rearrange("a b w -> (a b) w"))
            nc.sync.dma_start(out=ga_sb[:, :], in_=gamma.rearrange("c a b -> (a b) c"))
            # outer product ab[r*56+c] = alpha[r]*beta[c] on one partition
            ab_sb = wp.tile([1, H * W], F32)
            a_b = bass.AP(tensor=al_sb.tensor, offset=al_sb.offset,
                          ap=[list(al_sb.ap[0]), [1, H], [0, W]])
            b_b = bass.AP(tensor=be_sb.tensor, offset=be_sb.offset,
                          ap=[list(be_sb.ap[0]), [0, H], [1, W]])
            nc.vector.tensor_mul(ab_sb.rearrange("p (r c) -> p r c", c=W), a_b, b_b)
            # broadcast to 128 partitions via DRAM round trip
            ab_dram = dram.tile([1, H * W], F32)
            nc.sync.dma_start(out=ab_dram[:, :], in_=ab_sb[:, :])
            ab_src = bass.AP(tensor=ab_dram.tensor, offset=ab_dram.offset,
                             ap=[[0, 128], [1, H * W]])
            nc.sync.dma_start(out=ab_bcast[:, :], in_=ab_src)
            # gamma columns per cout chunk
            nc.sync.dma_start(out=ga_col[:, :],
                              in_=gamma.rearrange("(m p) a b -> p (m a b)", p=128))

        # ---------------- main loop ----------------
        xin = ctx.enter_context(tc.tile_pool(name="xin", bufs=4))
        outp = ctx.enter_context(tc.tile_pool(name="outp", bufs=3))
        mpsum = ctx.enter_context(tc.tile_pool(name="mpsum", bufs=8, space="PSUM"))

        for img in range(ipc):
            par = img % 2
            for k2 in range(2):
                xs = xin.tile([128, H * W], F32, name="xs", tag="xs")
                nc.sync.dma_start(out=xs[:, :],
                                  in_=x_flat[img, k2 * 128:(k2 + 1) * 128, :])
                # sign -> bf16 into padded interior (row stride 58)
                dst = ap3(xpad[par][k2], PW, H, 1, W, offset=ORIGIN + PW + 1)
                nc.scalar.sign(dst, xs.rearrange("p (h w) -> p h w", w=W))

            for m in range(2):
                osb = outp.tile([128, H * W], F32, name="osb", tag="osb")
                for blk in ((0, 4), (4, 7)):
                    pts = {}
                    for t in range(9):
                        dy, dx = t // 3, t % 3
                        for k2 in range(2):
                            slot = (t * 2 + k2) * 2 + m
                            lhsT = w_lhsT[:, slot * 128:(slot + 1) * 128]
                            first = (t == 0 and k2 == 0)
                            last = (t == 8 and k2 == 1)
                            for rc in range(*blk):
                                if first:
                                    pts[rc] = mpsum.tile([128, NMM], F32, name="pt",
                                                         tag="pt")
                                s = ORIGIN + (rc * NROW + dy) * PW + dx - 1
                                nc.tensor.matmul(
                                    pts[rc][:, :], lhsT,
                                    xpad[par][k2][:, s:s + NMM],
                                    start=first, stop=last,
                                )
                    for rc in range(*blk):
                        # (psum * gamma) * (alpha x beta), drop pad columns
                        pv = ap3(pts[rc], PW, NROW, 1, W, offset=1)
                        ov = ap3(osb, W, NROW, 1, W, offset=rc * NROW * W)
                        av = ap3(ab_bcast, W, NROW, 1, W, offset=rc * NROW * W)
                        nc.vector.scalar_tensor_tensor(
                            out=ov, in0=pv, scalar=ga_col[:, m:m + 1], in1=av,
                            op0=mybir.AluOpType.mult, op1=mybir.AluOpType.mult,
                        )
                nc.sync.dma_start(out=y_flat[img, m * 128:(m + 1) * 128, :],
                                  in_=osb[:, :])

    split_excess_waits(nc)
    return nc


_CACHE = {}


def _get_nc(ipc=IPC):
    key = ipc
    if key not in _CACHE:
        nc = bass.Bass("TRN2", target_bir_lowering=False, debug=False,
                       num_devices=1)
        _CACHE[key] = build(nc, ipc)
    return _CACHE[key]


def kernel(x, weight, alpha, beta, gamma):
    x = np.ascontiguousarray(np.asarray(x, dtype=np.float32))
    weight = np.ascontiguousarray(np.asarray(weight, dtype=np.float32))
    alpha = np.ascontiguousarray(np.asarray(alpha, dtype=np.float32))
    beta = np.ascontiguousarray(np.asarray(beta, dtype=np.float32))
    gamma = np.ascontiguousarray(np.asarray(gamma, dtype=np.float32))

    nc = _get_nc()
    in_maps = [
        {"x": x[i * IPC:(i + 1) * IPC], "w": weight,
         "alpha": alpha, "beta": beta, "gamma": gamma}
        for i in range(N_CORES)
    ]
    res = run_bass_kernel_spmd(nc, in_maps, core_ids=list(range(N_CORES)))
    return np.concatenate([res.results[i]["y"] for i in range(N_CORES)], axis=0)


# revision 23
# speedup vs baseline: 237.0849x; 237.0849x over previous
"""BinConv2d Trainium2 kernel.

Computes y = conv2d(sign(x), sign(w - mean_cin(w)), pad=1) * gamma * beta * alpha
for x (64,256,56,56) f32, w (256,256,3,3) f32, on 8 NeuronCores,
data-parallel over batch (8 images per core).

Strategy per core:
  - x image (256,56,56) f32 -> sign -> bf16, written into a zero-padded
    (58x58) layout in SBUF, split into 2 cin chunks of 128 partitions.
  - conv as 9 shifted matmuls per (cout chunk, 8-row chunk) accumulated in
    PSUM: psum[cout,pix] += wT[cin,cout](tap) @ xpad[cin, pix+shift(tap)].
  - weights: centered via a high-precision split summation (exact integer
    part + tiny residual) so binarized signs match the float64-exact signs
    (the jax reference's own rounding agrees with float64 on this data),
    then sign -> bf16, transposed on the tensor engine to [cin,cout] tiles.
  - psum evacuated with one scalar_tensor_tensor: (psum * gamma) * (alpha x beta),
    sliced to the valid 56 columns, DMA'd out as f32.
"""

import numpy as np
from contextlib import ExitStack

import concourse.bass as bass
import concourse.tile as tile
from concourse import mybir
from concourse.bass_utils import run_bass_kernel_spmd
from concourse.masks import make_identity

F32 = mybir.dt.float32
BF16 = mybir.dt.bfloat16
FP8 = mybir.dt.float8e4

N_CORES = 8
B, CIN, COUT, H, W, K = 64, 256, 256, 56, 56, 3
IPC = B // N_CORES          # images per core
PW = W + 2                  # padded row width (58)
NPAD = PW * PW + 12         # padded image buffer per cin chunk (+guard, align16)
ORIGIN = 1                  # index of padded (0,0) inside the buffer
NROW = 8                    # output rows per psum tile
NRC = H // NROW             # row chunks (7)
NMM = PW * NROW             # matmul free size (464)
USE_FP8 = True              # DoubleRow fp8 matmuls (2 MACs/cell/cycle)
XPAR = 4                    # sign(x) buffer parities (pipeline depth)


def split_excess_waits(nc, max_waits=1):
    """This container's walrus accepts at most one sync-wait per instruction;
    Tile's tail drain carries one wait per outstanding semaphore.  Split the
    extras into preceding single-wait EventSemaphore instructions (same
    engine, program order => identical semantics)."""
    for f in nc.m.functions:
        for bb in f.blocks:
            out = []
            for inst in bb.instructions:
                si = inst.sync_info
                if si is not None and si.on_wait and len(si.on_wait) > max_waits:
                    waits = list(si.on_wait)
                    extra, keep = waits[:-max_waits], waits[-max_waits:]
                    for w in extra:
                        n = mybir.InstEventSemaphore(
                            name=f"I-xw{nc.next_id()}",
                            ins=[],
                            outs=[],
                            sync_info=mybir.SyncInfo(on_wait=[w], on_update=[]),
                        )
                        n.engine = inst.engine
                        out.append(n)
                    si.on_wait = keep
                out.append(inst)
            bb.instructions = out


def ap3(t, outer_step, outer_n, inner_step, inner_n, offset=0):
    """[128p, outer, inner] view of a 2-D sbuf tile AP with custom steps."""
    return bass.AP(
        tensor=t.tensor,
        offset=t.offset + offset,
        ap=[list(t.ap[0]), [outer_step, outer_n], [inner_step, inner_n]],
    )


def build(nc, ipc=IPC, repeat=1):
    x = nc.dram_tensor("x", [ipc, CIN, H, W], F32, kind="ExternalInput").ap()
    wt = nc.dram_tensor("w", [COUT, CIN, K, K], F32, kind="ExternalInput").ap()
    alpha = nc.dram_tensor("alpha", [1, H, 1], F32, kind="ExternalInput").ap()
    beta = nc.dram_tensor("beta", [1, 1, W], F32, kind="ExternalInput").ap()
    gamma = nc.dram_tensor("gamma", [COUT, 1, 1], F32, kind="ExternalInput").ap()
    y = nc.dram_tensor("y", [ipc, COUT, H, W], F32, kind="ExternalOutput").ap()

    w_flat = wt.rearrange("co ci kh kw -> co (ci kh kw)")      # (256, 2304)
    x_flat = x.rearrange("b c h w -> b c (h w)")               # (ipc, 256, 3136)
    y_flat = y.rearrange("b c h w -> b c (h w)")               # (ipc, 256, 3136)

    with tile.TileContext(nc) as tc, ExitStack() as ctx:
        consts = ctx.enter_context(tc.tile_pool(name="consts", bufs=1))
        dram = ctx.enter_context(tc.tile_pool(name="dram", bufs=1, space="DRAM"))

        # ---------------- persistent tiles ----------------
        ident = consts.tile([128, 128], BF16)
        make_identity(nc, ident)

        # padded sign(x) buffers: [parity], cin chunk k at free offset k*NPAD
        XDT = FP8 if USE_FP8 else BF16
        xpad = [consts.tile([128, 2 * NPAD], XDT, name=f"xpad{p}")
                for p in range(XPAR)]
        for p in range(XPAR):
            for k in range(2):
                o = k * NPAD
                # zero only what matmuls can read and signs never write:
                # guard+top row, bottom row+tail, and the two pad columns
                nc.gpsimd.memset(xpad[p][:, o:o + ORIGIN + PW], 0.0)
                nc.gpsimd.memset(xpad[p][:, o + ORIGIN + 57 * PW:o + NPAD], 0.0)
                nc.gpsimd.memset(
                    ap3(xpad[p], PW, 57, 1, 2, offset=o + ORIGIN + 57), 0.0)

        if USE_FP8:
            # fp8 DoubleRow weights: per (tap, m) a [Ko=2, M=128] slot
            w8 = consts.tile([128, 9 * 2 * 256], FP8)
        else:
            w_lhsT = consts.tile([128, 36 * 128], BF16)  # [tap x k x m] tiles

        ab_bcast = consts.tile([128, H * W], F32)
        ga_col = consts.tile([128, 2], F32)

        # main-loop pools come first on the allocation stack: the wprep pool
        # is released before the main loop, and a later-allocated pool would
        # alias its addresses, adding a false WAR that stalls the x loads.
        xin = ctx.enter_context(tc.tile_pool(name="xin", bufs=5))
        outp = ctx.enter_context(tc.tile_pool(name="outp", bufs=3))
        mpsum = ctx.enter_context(tc.tile_pool(name="mpsum", bufs=8, space="PSUM"))

        # ---------------- weight preparation ----------------
        pps = mpsum  # transposes share the main psum pool's 8 bank slots
        with tc.tile_pool(name="wprep", bufs=1) as wp:
            wsign = []
            for m in range(2):
                w_st = wp.tile([128, 2304], F32, name=f"wst{m}")
                nc.scalar.dma_start(out=w_st[:, :], in_=w_flat[m * 128:(m + 1) * 128, :])

                # a = round(w * 2^22)  (exact integer part, sum is exact fp32)
                wa = wp.tile([128, 2304], F32, name=f"wa{m}")
                nc.scalar.activation(
                    out=wa[:, :], in_=w_st[:, :],
                    func=mybir.ActivationFunctionType.Copy,
                    bias=float(2.0 ** 23), scale=float(2.0 ** 22),
                )
                nc.vector.tensor_scalar_sub(wa[:, :], wa[:, :], float(2.0 ** 23))
                # r = w - a * 2^-22   (exact residual)
                wr = wp.tile([128, 2304], F32, name="wr", tag="wr")
                nc.vector.scalar_tensor_tensor(
                    out=wr[:, :], in0=wa[:, :], scalar=float(-(2.0 ** -22)),
                    in1=w_st[:, :], op0=mybir.AluOpType.mult, op1=mybir.AluOpType.add,
                )
                # reduce over cin (stride 9 view: [p, tap, cin])
                wSA = wp.tile([128, 16], F32, name=f"wSA{m}")
                wSr = wp.tile([128, 16], F32, name=f"wSr{m}")
                nc.vector.tensor_reduce(
                    out=wSA[:, 0:9], in_=ap3(wa, 1, 9, 9, 256),
                    axis=mybir.AxisListType.X, op=mybir.AluOpType.add,
                )
                nc.vector.tensor_reduce(
                    out=wSr[:, 0:9], in_=ap3(wr, 1, 9, 9, 256),
                    axis=mybir.AxisListType.X, op=mybir.AluOpType.add,
                )
                # mean_hi = SA * 2^-30 ; mean_lo = Sr / 256
                nc.scalar.mul(wSA[:, 0:9], wSA[:, 0:9], float(2.0 ** -30))
                nc.scalar.mul(wSr[:, 0:9], wSr[:, 0:9], float(1.0 / 256.0))
                # centered = (w - mean_hi) - mean_lo, written over wa
                # (wa's integer part is dead once SA and r are computed)
                for t in range(9):
                    vt = ap3(wa, 9, 256, 0, 1, offset=t)
                    st = ap3(w_st, 9, 256, 0, 1, offset=t)
                    eng = nc.vector if t % 2 == 0 else nc.gpsimd
                    eng.tensor_scalar_sub(vt, st, wSA[:, t:t + 1])
                    eng.tensor_scalar_sub(vt, vt, wSr[:, t:t + 1])
                ws = wp.tile([128, 2304], BF16, name=f"wsg{m}")
                nc.scalar.sign(ws[:, :], wa[:, :])
                wsign.append(ws)

            # transpose sign tiles to [cin, cout] per tap on the PE
            for t in range(9):
                for k2 in range(2):
                    for m in range(2):
                        slot = (t * 2 + k2) * 2 + m
                        src = ap3(wsign[m], 9, 128, 0, 1, offset=k2 * 128 * 9 + t)
                        pt = pps.tile([128, 128], BF16, name="tp", tag="pt")
                        nc.tensor.transpose(pt[:, :], src, ident[:, :])
                        if USE_FP8:
                            base = (t * 2 + m) * 256 + k2 * 128
                            nc.vector.tensor_copy(w8[:, base:base + 128], pt[:, :])
                        else:
                            nc.vector.tensor_copy(
                                w_lhsT[:, slot * 128:(slot + 1) * 128], pt[:, :])

            # ---------------- scale tensors ----------------
            al_sb = wp.tile([1, 64], F32)
            be_sb = wp.tile([1, 64], F32)
            ga_sb = wp.tile([1, 256], F32)
            nc.scalar.dma_start(out=al_sb[:, 0:H], in_=alpha.rearrange("a h b -> (a b) h"))
            nc.scalar.dma_start(out=be_sb[:, 0:W], in_=beta.rearrange("a b w -> (a b) w"))
            nc.scalar.dma_start(out=ga_sb[:, :], in_=gamma.rearrange("c a b -> (a b) c"))
            # outer product ab[r*56+c] = alpha[r]*beta[c], staged in row 0
            # of ab_bcast itself (broadcast below overwrites all rows)
            ab_sb = ab_bcast[0:1, :]
            a_b = bass.AP(tensor=al_sb.tensor, offset=al_sb.offset,
                          ap=[list(al_sb.ap[0]), [1, H], [0, W]])
            b_b = bass.AP(tensor=be_sb.tensor, offset=be_sb.offset,
                          ap=[list(be_sb.ap[0]), [0, H], [1, W]])
            nc.vector.tensor_mul(ab_sb.rearrange("p (r c) -> p r c", c=W), a_b, b_b)
            # broadcast row 0 to all 128 partitions with a K=1 ones-matmul
            # (values here are exact: alpha/beta are ones; avoids 1.6 MB of DMA)
            ones_col = wp.tile([1, 128], F32)
            nc.vector.memset(ones_col[:, :], 1.0)
            for ci in range(NRC):
                cs = ci * NROW * W
                ps_ab = pps.tile([128, NMM], F32, name="ps_ab", tag="pt")
                nc.tensor.matmul(ps_ab[:, 0:NROW * W], ones_col[:, :],
                                 ab_sb[:, cs:cs + NROW * W])
                nc.vector.tensor_copy(ab_bcast[:, cs:cs + NROW * W],
                                      ps_ab[:, 0:NROW * W])
            # gamma columns per cout chunk
            nc.scalar.dma_start(out=ga_col[:, :],
                              in_=gamma.rearrange("(m p) a b -> p (m a b)", p=128))

        # ---------------- main loop ----------------
        if repeat > 1:
            rep_cm = tc.For_i(0, repeat, 1)
            rep_cm.__enter__()

        for img in range(ipc):
            par = img % XPAR
            for k2 in range(2):
                xs = xin.tile([128, H * W], F32, name="xs", tag="xs")
                nc.sync.dma_start(out=xs[:, :],
                                  in_=x_flat[img, k2 * 128:(k2 + 1) * 128, :])
                # sign -> xdt into padded interior (row stride 58)
                dst = ap3(xpad[par], PW, H, 1, W, offset=k2 * NPAD + ORIGIN + PW + 1)
                nc.scalar.sign(dst, xs.rearrange("p (h w) -> p h w", w=W))

            for m in range(2):
                osb = outp.tile([128, H * W], F32, name="osb", tag="osb")
                for blk in ((0, 4), (4, 7)):
                    pts = {}
                    for t in range(9):
                        dy, dx = t // 3, t % 3
                        if USE_FP8:
                            lhsT = ap3(w8, 128, 2, 1, 128, offset=(t * 2 + m) * 256)
                            first, last = (t == 0), (t == 8)
                            for rc in range(*blk):
                                if first:
                                    pts[rc] = mpsum.tile([128, NMM], F32, name="pt",
                                                         tag="pt")
                                s = ORIGIN + (rc * NROW + dy) * PW + dx - 1
                                rhs = ap3(xpad[par], NPAD, 2, 1, NMM, offset=s)
                                nc.tensor.matmul(
                                    pts[rc][:, :], lhsT, rhs,
                                    start=first, stop=last,
                                    perf_mode=mybir.MatmulPerfMode.DoubleRow,
                                )
                            continue
                        for k2 in range(2):
                            slot = (t * 2 + k2) * 2 + m
                            lhsT = w_lhsT[:, slot * 128:(slot + 1) * 128]
                            first = (t == 0 and k2 == 0)
                            last = (t == 8 and k2 == 1)
                            for rc in range(*blk):
                                if first:
                                    pts[rc] = mpsum.tile([128, NMM], F32, name="pt",
                                                         tag="pt")
                                s = ORIGIN + (rc * NROW + dy) * PW + dx - 1
                                nc.tensor.matmul(
                                    pts[rc][:, :], lhsT,
                                    xpad[par][:, k2 * NPAD + s:k2 * NPAD + s + NMM],
                                    start=first, stop=last,
                                )
                    for rc in range(*blk):
                        # (psum * gamma) * (alpha x beta), drop pad columns
                        pv = ap3(pts[rc], PW, NROW, 1, W, offset=1)
                        ov = ap3(osb, W, NROW, 1, W, offset=rc * NROW * W)
                        av = ap3(ab_bcast, W, NROW, 1, W, offset=rc * NROW * W)
                        nc.vector.scalar_tensor_tensor(
                            out=ov, in0=pv, scalar=ga_col[:, m:m + 1], in1=av,
                            op0=mybir.AluOpType.mult, op1=mybir.AluOpType.mult,
                        )
                # store on the ACT HWDGE ring (input loads use the SP ring;
                # separate rings pipeline independently)
                nc.scalar.dma_start(out=y_flat[img, m * 128:(m + 1) * 128, :],
                                    in_=osb[:, :])

        if repeat > 1:
            rep_cm.__exit__(None, None, None)

    split_excess_waits(nc)
    return nc


_CACHE = {}


def _get_nc(ipc=IPC):
    key = ipc
    if key not in _CACHE:
        nc = bass.Bass("TRN2", target_bir_lowering=False, debug=False,
                       num_devices=1)
        _CACHE[key] = build(nc, ipc)
    return _CACHE[key]


def kernel(x, weight, alpha, beta, gamma):
    x = np.ascontiguousarray(np.asarray(x, dtype=np.float32))
    weight = np.ascontiguousarray(np.asarray(weight, dtype=np.float32))
    alpha = np.ascontiguousarray(np.asarray(alpha, dtype=np.float32))
    beta = np.ascontiguousarray(np.asarray(beta, dtype=np.float32))
    gamma = np.ascontiguousarray(np.asarray(gamma, dtype=np.float32))

    nc = _get_nc()
    in_maps = [
        {"x": x[i * IPC:(i + 1) * IPC], "w": weight,
         "alpha": alpha, "beta": beta, "gamma": gamma}
        for i in range(N_CORES)
    ]
    res = run_bass_kernel_spmd(nc, in_maps, core_ids=list(range(N_CORES)))
    return np.concatenate([res.results[i]["y"] for i in range(N_CORES)], axis=0)
